# revision 2
# baseline (speedup 1.0000x reference)
"""Trainium2 Bass kernel for DilatedSpatialAttention — v2 (PE-packed).

Problem: B=16, H=W=32, C=256, heads=8, head_dim=32, depthwise 3x3
dilation-2 SAME conv on key/value, softmax attention over S=1024.
Sharding: data-parallel over batch, 2 batches/core.

Design:
  - Scores: 16-tile PE packing (4 heads x 4 kpos-blocks of 32) per
    (kt, half, qb); psum split in two [128, 2, 512] tiles (head-sets
    {0,1} / {2,3}) so exp can be split/pipelined per head-set.
  - exp: one N=1024 ACTIVATE per head-set tile.
  - Software-pipelined PE order: AV(kt-1) is emitted after
    scores(kt) so the in-order PE stream never blocks on exp(kt).
  - Conv: 16-tile packing via rotation Latin square: tile (i,j)
    handles job J=(i-j)%4 (J: 0=k/h0 1=k/h1 2=v/h0 3=v/h1), chunk j;
    xpad for job J stores chunk c at row group (c+J)%4, baked in by
    rotated-channel input DMA segments (all positive strides).
  - AV: 2x(M=33) column packing, denominator via ones-row in vaug.
  - Input DMAs consolidated 2 kt per transfer; output 4 row-tiles
    per transfer.
"""

import numpy as np

B, H, W, C = 16, 32, 32, 256
HEADS = 8
HD = C // HEADS            # 32
KSZ, DIL = 3, 2
SCALE = float(HD) ** -0.5
NCORES = 8
BPC = B // NCORES          # batches per core
S = H * W                  # 1024
NKT = S // 128             # 8 kpos tiles

_CACHE = {}

# input DMA channel segments per job J (rotation (c+J)%4):
# SBUF 32-block g holds dram chunk (g-J)%4  ->  (sbuf_off, dram_off, width)
PERM_SEGS = {
    0: [(0, 0, 128)],
    1: [(0, 96, 32), (32, 0, 96)],
    2: [(0, 64, 64), (64, 0, 64)],
    3: [(0, 32, 96), (96, 0, 32)],
}


def _build(nc, tile, bass, mybir, repeat=None):
    from contextlib import ExitStack
    from concourse.masks import make_identity

    f32 = mybir.dt.float32
    bf16 = mybir.dt.bfloat16

    q_d = nc.dram_tensor("query", [BPC, S, C], f32, kind="ExternalInput")
    k_d = nc.dram_tensor("key_in", [BPC, S, C], f32, kind="ExternalInput")
    v_d = nc.dram_tensor("value", [BPC, S, C], f32, kind="ExternalInput")
    ck_d = nc.dram_tensor("conv_kernel", [KSZ * KSZ, C], f32, kind="ExternalInput")
    cb_d = nc.dram_tensor("conv_bias", [C], f32, kind="ExternalInput")
    out_d = nc.dram_tensor("out", [BPC, S, C], f32, kind="ExternalOutput")

    HP = H + 2 * DIL  # 36
    WP = W + 2 * DIL  # 36

    with ExitStack() as ctx:
        tc = ctx.enter_context(tile.TileContext(nc))
        const = ctx.enter_context(tc.tile_pool(name="const", bufs=1))
        sin = ctx.enter_context(tc.tile_pool(name="sin", bufs=3))
        sbf_p = ctx.enter_context(tc.tile_pool(name="sbf", bufs=3))
        stg_p = ctx.enter_context(tc.tile_pool(name="stg", bufs=6))
        xpad_p = ctx.enter_context(tc.tile_pool(name="xpad", bufs=8))
        qc_p = ctx.enter_context(tc.tile_pool(name="qc", bufs=4))
        kc_p = ctx.enter_context(tc.tile_pool(name="kc", bufs=4))
        vc_p = ctx.enter_context(tc.tile_pool(name="vc", bufs=4))
        vaug_p = ctx.enter_context(tc.tile_pool(name="vaug", bufs=16))
        p_p = ctx.enter_context(tc.tile_pool(name="pp", bufs=6))
        ot_p = ctx.enter_context(tc.tile_pool(name="ot", bufs=2))
        orow_p = ctx.enter_context(tc.tile_pool(name="orow", bufs=4))
        small_p = ctx.enter_context(tc.tile_pool(name="small", bufs=8))
        # PSUM: 2x2 banks scores/conv + 2x1 transpose + 2x1 AV accum
        scp = ctx.enter_context(tc.tile_pool(name="scp", bufs=2, space="PSUM"))
        ppp = ctx.enter_context(tc.tile_pool(name="ppp", bufs=2, space="PSUM"))
        acc_p = ctx.enter_context(tc.tile_pool(name="accp", bufs=2, space="PSUM"))

        # ---- constants ----
        ident = const.tile([128, 128], f32)
        make_identity(nc, ident[:])
        identb = const.tile([128, 128], bf16)
        nc.vector.tensor_copy(out=identb[:], in_=ident[:])

        # Latin-square diagonal conv weights:
        # tile (i,j) => job J=(i-j)%4, chunk j:
        #   wd[32i+r, j, tap, c] = ck[tap, half(J)*128 + 32j + c] * (r==c)
        wd = const.tile([128, 4, KSZ * KSZ, 32], f32)
        for i in range(4):
            for j in range(4):
                half = ((i - j) % 4) & 1
                src = bass.AP(
                    ck_d, half * 128 + 32 * j,
                    [[0, 32], [C, KSZ * KSZ], [1, 32]],
                )
                nc.gpsimd.dma_start(out=wd[32 * i:32 * i + 32, j], in_=src)
                nc.gpsimd.affine_select(
                    out=wd[32 * i:32 * i + 32, j],
                    in_=wd[32 * i:32 * i + 32, j],
                    compare_op=mybir.AluOpType.is_equal,
                    fill=0.0,
                    base=0,
                    pattern=[[0, KSZ * KSZ], [-1, 32]],
                    channel_multiplier=1,
                )
        wdb = const.tile([128, 4, KSZ * KSZ, 32], bf16)
        nc.vector.tensor_copy(out=wdb[:], in_=wd[:])

        bias_c = const.tile([128, 2], f32)
        for half in range(2):
            nc.gpsimd.dma_start(
                out=bias_c[:, half:half + 1],
                in_=bass.AP(cb_d, half * 128, [[1, 128], [1, 1]]),
            )

        rep_ctx = tc.For_i(0, repeat, 1) if repeat else None
        if rep_ctx is not None:
            ctx.enter_context(rep_ctx)

        state = {}

        def prep_chunks(b, use_pe=False):
            """Closures for batch b's prep; fills state[b].
            Returns (light_chunks, heavy_chunks): heavy = conv+vaug
            (contend for scores psum / depend on conv).
            use_pe: transpose via PE+DVE (for batch 0, when PE is
            idle) instead of the DMA XBAR (sync-queue heavy)."""
            qc = [qc_p.tile([128, S], bf16, tag="qc", name="qc") for _ in range(2)]
            kc = [kc_p.tile([128, S], bf16, tag="kc", name="kc") for _ in range(2)]
            vc = [vc_p.tile([128, S], bf16, tag="vc", name="vc") for _ in range(2)]
            vaug = [vaug_p.tile([128, HEADS * (HD + 1)], bf16, tag="va",
                                name="va") for _ in range(NKT)]
            xpad = {}
            state[b] = (qc, kc, vaug)
            light = []
            heavy = []

            def mk_xpad():
                for J in range(4):
                    xp = xpad_p.tile([128, HP, WP], bf16, tag="xpad",
                                     name="xpad")
                    nc.vector.memset(xp[:], 0.0)
                    xpad[J] = xp

            def load_st(tname, dram, kt4):
                st = sin.tile([128, 4, C], f32, tag="sin", name="st")
                if tname == "q":
                    nc.sync.dma_start(
                        out=st[:],
                        in_=bass.AP(dram, b * S * C + kt4 * 512 * C,
                                    [[C, 128], [128 * C, 4], [1, C]]))
                else:
                    for half in range(2):
                        J = (0 if tname == "k" else 2) + half
                        for s_off, d_off, wdt in PERM_SEGS[J]:
                            nc.sync.dma_start(
                                out=st[:, :, 128 * half + s_off:
                                       128 * half + s_off + wdt],
                                in_=bass.AP(
                                    dram,
                                    b * S * C + kt4 * 512 * C
                                    + 128 * half + d_off,
                                    [[C, 128], [128 * C, 4], [1, wdt]]))
                return st

            def store_block(tname, half, kt, src_get):
                """src_get(dst_bf16_[128,128]-writer) per transposed blk"""
                if tname == "q":
                    return qc[half][:, kt * 128:(kt + 1) * 128], None
                J = (0 if tname == "k" else 2) + half
                dst = xpad[J][:, DIL + 4 * kt:DIL + 4 * kt + 4, DIL:DIL + W]
                return None, dst

            st_slots = {}

            def mk_trans_dma(tname, dram, kt4):
                def go():
                    st_slots[(tname, kt4)] = load_st(tname, dram, kt4)
                return go

            def mk_trans_part(tname, kt):
                """One kt block: 2 PE transposes + 2 evac copies."""
                def go():
                    st = st_slots[(tname, kt // 4)]
                    t = kt % 4
                    pt = ppp.tile([128, 512], f32, tag="pp", name="pt")
                    for half in range(2):
                        nc.tensor.transpose(
                            pt[:, 128 * half:128 * half + 128],
                            st[:, t, 128 * half:128 * half + 128],
                            ident[:])
                        qdst, xdst = store_block(tname, half, kt, None)
                        if qdst is not None:
                            nc.vector.tensor_copy(
                                out=qdst,
                                in_=pt[:, 128 * half:128 * half + 128])
                        else:
                            nc.vector.tensor_copy(
                                out=xdst,
                                in_=pt[:, 128 * half:128 * half + 128]
                                .rearrange("p (r w) -> p r w", w=W))
                return go

            cv_slots = {}

            def mk_conv_part(jp, sb, tap_lo, tap_hi):
                """Conv quantum: taps [tap_lo, tap_hi) of the 8-tile
                span for job-pair jp (0: k, 1: v), s-block sb. Uses
                ppp psum so it never blocks the scores pipeline."""
                def go():
                    if tap_lo == 0:
                        cv_slots[(jp, sb)] = [
                            ppp.tile([128, 512], f32, tag="pp", name="cv")
                            for _ in range(2)]
                    cvs = cv_slots[(jp, sb)]
                    for tap in range(tap_lo, tap_hi):
                        dh, dw = divmod(tap, KSZ)
                        for i in range(4):
                            for j in range(4):
                                J = (i - j) % 4
                                if J // 2 != jp:
                                    continue
                                nc.tensor.matmul(
                                    out=cvs[J % 2][32 * j:32 * j + 32, :],
                                    lhsT=wdb[32 * i:32 * i + 32, j, tap, :],
                                    rhs=xpad[J][32 * i:32 * i + 32,
                                                16 * sb + DIL * dh:
                                                16 * sb + DIL * dh + 16,
                                                DIL * dw:DIL * dw + W],
                                    start=(tap == 0),
                                    stop=(tap == KSZ * KSZ - 1),
                                    tile_position=(32 * i, 32 * j))
                    if tap_hi == KSZ * KSZ:
                        for J in (2 * jp, 2 * jp + 1):
                            half = J & 1
                            dstt = kc[half] if J < 2 else vc[half]
                            nc.vector.tensor_scalar_add(
                                out=dstt[:, sb * 512:(sb + 1) * 512],
                                in0=cvs[J % 2][:],
                                scalar1=bias_c[:, half:half + 1])
                return go

            def mk_vaug(kt):
                def go():
                    va3 = vaug[kt][:].rearrange("p (h x) -> p h x", x=HD + 1)
                    nc.vector.memset(va3[:, :, HD:HD + 1], 1.0)
                    pt = ppp.tile([128, 512], bf16, tag="pp", name="pt")
                    for half in range(2):
                        nc.tensor.transpose(
                            pt[:, 128 * half:128 * half + 128],
                            vc[half][:, kt * 128:(kt + 1) * 128],
                            identb[:])
                        nc.vector.tensor_copy(
                            out=va3[:, 4 * half:4 * half + 4, 0:HD],
                            in_=pt[:, 128 * half:128 * half + 128]
                            .rearrange("p (h d) -> p h d", d=HD))
                return go

            def trans_quanta(tname, dram):
                qs = []
                for kt4 in range(2):
                    qs.append(mk_trans_dma(tname, dram, kt4))
                    for t in range(4):
                        qs.append(mk_trans_part(tname, 4 * kt4 + t))
                return qs

            def conv_quanta(jp):
                qs = []
                for sb in range(2):
                    for lo, hi in ((0, 3), (3, 6), (6, 9)):
                        qs.append(mk_conv_part(jp, sb, lo, hi))
                return qs

            light.append(mk_xpad)
            light.extend(trans_quanta("k", k_d))
            light.extend(conv_quanta(0))
            light.extend(trans_quanta("q", q_d))
            light.extend(trans_quanta("v", v_d))
            light.extend(conv_quanta(1))
            for kt in range(NKT):
                light.append(mk_vaug(kt))
            return light, heavy

        def attn_units(b):
            qc, kc, vaug = state[b]
            units = []

            def mk_unit(half, qb):
                q0 = qb * 512

                def emit_scores(kt):
                    # 4 concurrent M=128 matmuls, one per head (row
                    # groups 32i), each filling its own psum bank
                    ps = []
                    for hs in range(2):     # head-set {0,1} / {2,3}
                        sc = scp.tile([128, 2, 512], f32, tag="sc", name="sc")
                        for ii in range(2):
                            i = 2 * hs + ii
                            nc.tensor.matmul(
                                out=sc[:, ii, :],
                                lhsT=kc[half][32 * i:32 * i + 32,
                                              kt * 128:(kt + 1) * 128],
                                rhs=qc[half][32 * i:32 * i + 32,
                                             q0:q0 + 512],
                                start=True, stop=True,
                                tile_position=(32 * i, 0))
                        p = p_p.tile([128, 2, 512], bf16, tag="p", name="p")
                        nc.scalar.activation(
                            out=p[:], in_=sc[:],
                            func=mybir.ActivationFunctionType.Exp,
                            scale=SCALE)
                        ps.append(p)
                    return ps

                def emit_av(accs, kt, ps):
                    for pr in range(2):
                        for jj in range(2):
                            g = half * 4 + pr * 2 + jj
                            nc.tensor.matmul(
                                out=accs[pr][64 * jj:64 * jj + HD + 1, :],
                                lhsT=vaug[kt][:, (HD + 1) * g:
                                              (HD + 1) * g + HD + 1],
                                rhs=ps[pr][:, jj, :],
                                start=(kt == 0), stop=(kt == NKT - 1))

                def go(fillers=(), end_fillers=()):
                    fillers = list(fillers)
                    nf = len(fillers)
                    accs = [acc_p.tile([128, 512], f32, tag="acc", name="acc")
                            for _ in range(2)]
                    # software pipeline: AV(kt-1) emitted after scores(kt)
                    prev = None
                    for kt in range(NKT):
                        lo = kt * nf // NKT
                        hi = (kt + 1) * nf // NKT
                        for f in fillers[lo:hi]:
                            f()
                        ps = emit_scores(kt)
                        if prev is not None:
                            emit_av(accs, kt - 1, prev)
                        prev = ps
                    emit_av(accs, NKT - 1, prev)
                    for f in end_fillers:
                        f()
                    # normalize + output
                    for pr in range(2):
                        g0 = half * 4 + pr * 2
                        ot = ot_p.tile([128, 512], f32, tag="ot", name="ot")
                        nc.vector.tensor_copy(out=ot[0:97, :],
                                              in_=accs[pr][0:97, :])
                        otile = orow_p.tile([128, 4, 2 * HD], f32,
                                            tag="orow", name="orow")
                        for u in range(4):
                            tp = ppp.tile([128, 512], f32, tag="pp", name="pt")
                            nc.tensor.transpose(
                                tp[:, 0:97], ot[0:97, u * 128:(u + 1) * 128],
                                ident[0:97, 0:97])
                            rc = small_p.tile([128, 2], f32, tag="rc",
                                              name="rc")
                            sums = bass.AP(tp.tensor, tp.offset + HD,
                                           [tp.ap[0], [64, 2]])
                            nc.vector.reciprocal(rc[:], sums)
                            for jj in range(2):
                                nc.vector.tensor_scalar_mul(
                                    out=otile[:, u, HD * jj:HD * jj + HD],
                                    in0=tp[:, 64 * jj:64 * jj + HD],
                                    scalar1=rc[:, jj:jj + 1])
                        nc.sync.dma_start(
                            out=bass.AP(
                                out_d,
                                b * S * C + qb * 512 * C + HD * g0,
                                [[C, 128], [128 * C, 4], [1, 2 * HD]]),
                            in_=otile[:])
                return go

            for half in range(2):
                for qb in range(2):
                    units.append(mk_unit(half, qb))
            return units

        # emission: prep(0) inline, then attn(b) with prep(b+1)'s
        # quanta spread evenly across all 4 units' kt slots.
        light, heavy = prep_chunks(0, use_pe=True)
        for ch in light + heavy:
            ch()
        for b in range(BPC):
            units = attn_units(b)
            if b + 1 < BPC:
                light, heavy = prep_chunks(b + 1, use_pe=True)
            else:
                light, heavy = [], []
            n_l = len(light)
            n_u = len(units)
            for i, unit in enumerate(units):
                lo = i * n_l // n_u
                hi = (i + 1) * n_l // n_u
                unit(fillers=light[lo:hi])

    return nc


def _get_nc():
    if "nc" not in _CACHE:
        import concourse.bass as bass
        import concourse.tile as tile
        from concourse import bacc, mybir

        nc = bacc.Bacc("TRN2", target_bir_lowering=False, debug=False)
        _build(nc, tile, bass, mybir)
        nc.compile()
        _CACHE["nc"] = nc
    return _CACHE["nc"]


def kernel(**inputs):
    q = np.ascontiguousarray(
        np.asarray(inputs["query"], dtype=np.float32).reshape(B, S, C))
    k = np.ascontiguousarray(
        np.asarray(inputs["key_in"], dtype=np.float32).reshape(B, S, C))
    v = np.ascontiguousarray(
        np.asarray(inputs["value"], dtype=np.float32).reshape(B, S, C))
    ck = np.ascontiguousarray(
        np.asarray(inputs["conv_kernel"], dtype=np.float32).reshape(
            KSZ * KSZ, C))
    cb = np.ascontiguousarray(
        np.asarray(inputs["conv_bias"], dtype=np.float32).reshape(C))

    in_maps = []
    for i in range(NCORES):
        lo, hi = i * BPC, (i + 1) * BPC
        in_maps.append({
            "query": np.ascontiguousarray(q[lo:hi]),
            "key_in": np.ascontiguousarray(k[lo:hi]),
            "value": np.ascontiguousarray(v[lo:hi]),
            "conv_kernel": ck,
            "conv_bias": cb,
        })

    from concourse.bass_utils import run_bass_kernel_spmd

    nc = _get_nc()
    res = run_bass_kernel_spmd(
        nc, in_maps, core_ids=list(range(NCORES)),
        **_CACHE.get("run_kwargs", {}),
    )
    _CACHE["last_result"] = res
    out = np.concatenate([r["out"] for r in res.results], axis=0)
    return out.reshape(B, H, W, C)
